# revision 3
# baseline (speedup 1.0000x reference)
"""GQA multi-head attention (B=2, S=2048, D=2048, HQ=16, HKV=4, DK=128) with
RoPE + causal softmax + output projection, sharded over 8 NeuronCores as
(batch x kv-head-group): core c handles batch c//4, kv head c%4 (4 query
heads). w_q/w_kv column-sharded, fc row-sharded; partial fc outputs are
summed on the host (the "all-reduce").

Schedule: phase 1 streams xT over two DMA rings (sync/vector) while wq
streams on the scalar ring and misc weights on the gpsimd ring, feeding
kc-outer Q/K/V projection matmuls. Phase 2 runs attention heads in qc
order (1,2,3,0) with a fine-grained FIFO of fc-projection matmul chunks
drained into the attention pipeline's latency bubbles; softmax
denominators use 4-way-summed exp tiles (DVE pair adds + gpsimd combine)
to quarter the PE ones-matmul columns, and PV matmuls skip the dead
columns of diagonal tiles. Partial fc outputs are written in fp16.
"""

import sys
from collections import deque

for _p in ("/opt/trn_rl_repo", "/root/.axon_site", "/root/.axon_site/_ro/trn_rl_repo"):
    if _p not in sys.path:
        sys.path.insert(0, _p)

import numpy as np

import concourse.bass as bass
import concourse.mybir as mybir
import concourse.tile as tile
from concourse import bacc
from concourse.bass_utils import run_bass_kernel_spmd

F32 = mybir.dt.float32
F16 = mybir.dt.float16

B, S, D = 2, 2048, 2048
HKV, NREP, DK = 4, 4, 128
HG = NREP  # query heads per core
KC = D // 128  # contraction chunks
SQC = S // 512  # 512-wide query column chunks
SCALE = float(1.0 / np.sqrt(DK))

_COMPILED = None


def _build():
    nc = bacc.Bacc(None, target_bir_lowering=False, debug=False)

    xT = nc.dram_tensor("xT", [D, S], F16, kind="ExternalInput")
    wq = nc.dram_tensor("wq", [D, HG * DK], F16, kind="ExternalInput")
    wk = nc.dram_tensor("wk", [D, DK], F16, kind="ExternalInput")
    wv = nc.dram_tensor("wv", [D, DK], F16, kind="ExternalInput")
    fcw = nc.dram_tensor("fcw", [HG * DK, D], F16, kind="ExternalInput")
    cosT = nc.dram_tensor("cosT", [128, S], F16, kind="ExternalInput")
    sinT = nc.dram_tensor("sinT", [128, S], F16, kind="ExternalInput")
    masks = nc.dram_tensor("masks", [128, 4, 512], F16, kind="ExternalInput")
    onesc = nc.dram_tensor("onesc", [128, 1], F16, kind="ExternalInput")
    iden = nc.dram_tensor("iden", [128, 128], F16, kind="ExternalInput")
    out = nc.dram_tensor("out", [S, D], F16, kind="ExternalOutput")

    with tile.TileContext(nc) as tc:
        with tc.tile_pool(name="persist", bufs=1) as persist:
            # attention-phase residents
            qt_sb = persist.tile([128, HG, S], F16)  # Q^T, rope'd, per head
            kt_sb = persist.tile([128, S], F16)  # K^T rope'd
            v_sb = persist.tile([128, KC, DK], F16)  # V  [sk, dk] chunks
            ctxT = persist.tile([128, HG, S], F16)  # (softmax @ V)^T per head
            cos_sb = persist.tile([128, S], F16)
            sin_sb = persist.tile([128, S], F16)
            mask_sb = persist.tile([128, 4, 512], F16)
            ones_sb = persist.tile([128, 1], F16)
            iden_sb = persist.tile([128, 128], F16)
            fcw_sb = persist.tile([128, HG, D], F16)

            # pools shared across all phases (no release/realloc barriers)
            ps8 = tc.alloc_tile_pool(name="ps8", bufs=8, space="PSUM")
            es_pool = tc.alloc_tile_pool(name="es_pool", bufs=6)
            nrm_pool = tc.alloc_tile_pool(name="nrm_pool", bufs=3)

            with tc.tile_pool(name="p1sb", bufs=1) as p1sb, \
                 tc.tile_pool(name="p1tmp", bufs=2) as p1tmp:
                xt_sb = p1sb.tile([128, KC, S], F16)
                wq_sb = p1sb.tile([128, KC, HG * DK], F16)
                wk_sb = p1sb.tile([128, KC, DK], F16)
                wv_sb = p1sb.tile([128, KC, DK], F16)
                vt_sb = p1sb.tile([128, S], F16)

                # DMA rings balanced by phase-1 need-time: scalar carries wq
                # (consumed at ~2us/chunk with xT) then the rope/K/V-stage
                # weights; xT is split across the sync and gpsimd rings
                # (vector can't issue DMAs); fcw rides gpsimd last.
                wqr = wq.rearrange("(k p) m -> p k m", p=128)
                xr = xT.rearrange("(k p) s -> p k s", p=128)
                for k in range(KC):
                    nc.scalar.dma_start(out=wq_sb[:, k, :], in_=wqr[:, k, :])
                    eng = nc.sync if k % 2 == 0 else nc.gpsimd
                    eng.dma_start(out=xt_sb[:, k, :], in_=xr[:, k, :])
                nc.scalar.dma_start(out=cos_sb, in_=cosT[:])
                nc.scalar.dma_start(out=sin_sb, in_=sinT[:])
                nc.scalar.dma_start(out=wk_sb, in_=wk.rearrange("(k p) m -> p k m", p=128))
                nc.scalar.dma_start(out=wv_sb, in_=wv.rearrange("(k p) m -> p k m", p=128))
                nc.scalar.dma_start(out=iden_sb, in_=iden[:])
                nc.scalar.dma_start(out=mask_sb, in_=masks[:])
                nc.scalar.dma_start(out=ones_sb, in_=onesc[:])
                nc.gpsimd.dma_start(out=fcw_sb, in_=fcw.rearrange("(h p) n -> p h n", p=128))

                def rope_full(dst, tq):
                    # dst/tq: [128, S] fp16; evens in partitions 0:64, odds 64:128.
                    # cos/sin are duplicated across both halves so every
                    # SBUF*SBUF tensor op has equal input base partitions.
                    pe, po = tq[0:64, :], tq[64:128, :]
                    t1 = p1tmp.tile([64, S], F16, name="t1", tag="t1")
                    t2 = p1tmp.tile([64, S], F16, name="t2", tag="t2")
                    nc.vector.tensor_tensor(t1, pe, cos_sb[0:64, :], op=mybir.AluOpType.mult)
                    nc.vector.tensor_tensor(t2, po, sin_sb[64:128, :], op=mybir.AluOpType.mult)
                    nc.vector.tensor_tensor(dst[0:64, :], t1, t2, op=mybir.AluOpType.subtract)
                    t3 = p1tmp.tile([64, S], F16, name="t3", tag="t1")
                    t4 = p1tmp.tile([64, S], F16, name="t4", tag="t2")
                    nc.vector.tensor_tensor(t3, pe, sin_sb[0:64, :], op=mybir.AluOpType.mult)
                    nc.vector.tensor_tensor(t4, po, cos_sb[64:128, :], op=mybir.AluOpType.mult)
                    nc.vector.tensor_tensor(dst[64:128, :], t3, t4, op=mybir.AluOpType.add)

                # Q^T = wq^T @ xT, kc-outer so PE consumes chunks as they land
                for half in range(2):
                    accs = []
                    for mh in (2 * half, 2 * half + 1):
                        for qc in range(SQC):
                            psq = ps8.tile([128, 512], F32, name="psq", tag="pp")
                            accs.append((mh, qc, psq))
                    for k in range(KC):
                        for mh, qc, psq in accs:
                            nc.tensor.matmul(psq, wq_sb[:, k, mh * 128:(mh + 1) * 128],
                                             xt_sb[:, k, qc * 512:(qc + 1) * 512],
                                             start=(k == 0), stop=(k == KC - 1))
                    tqs = {}
                    for mh in (2 * half, 2 * half + 1):
                        tqs[mh] = p1tmp.tile([128, S], F16, name="tq", tag="tq")
                    for mh, qc, psq in accs:
                        nc.scalar.copy(tqs[mh][:, qc * 512:(qc + 1) * 512], psq)
                    for mh in (2 * half, 2 * half + 1):
                        rope_full(qt_sb[:, mh, :], tqs[mh])

                # K^T = wk^T @ xT
                kaccs = [ps8.tile([128, 512], F32, name="psk", tag="pp")
                         for _ in range(SQC)]
                for k in range(KC):
                    for qc in range(SQC):
                        nc.tensor.matmul(kaccs[qc], wk_sb[:, k, :],
                                         xt_sb[:, k, qc * 512:(qc + 1) * 512],
                                         start=(k == 0), stop=(k == KC - 1))
                tk = p1tmp.tile([128, S], F16, name="tk", tag="tq")
                for qc in range(SQC):
                    nc.scalar.copy(tk[:, qc * 512:(qc + 1) * 512], kaccs[qc])
                rope_full(kt_sb, tk)

                # V^T = wv^T @ xT (N=512), then PE-transpose to V [sk, dk]
                vaccs = [ps8.tile([128, 512], F32, name="psvt", tag="pp")
                         for _ in range(SQC)]
                for k in range(KC):
                    for sc in range(SQC):
                        nc.tensor.matmul(vaccs[sc], wv_sb[:, k, :],
                                         xt_sb[:, k, sc * 512:(sc + 1) * 512],
                                         start=(k == 0), stop=(k == KC - 1))
                for sc in range(SQC):
                    nc.scalar.copy(vt_sb[:, sc * 512:(sc + 1) * 512], vaccs[sc])
                for gq in range(4):
                    psv = ps8.tile([128, 512], F16, name="psv", tag="pp")
                    for vt in range(4):
                        skt = gq * 4 + vt
                        nc.tensor.matmul(psv[:, vt * 128:(vt + 1) * 128],
                                         vt_sb[:, skt * 128:(skt + 1) * 128],
                                         iden_sb, is_transpose=True,
                                         start=True, stop=True)
                    nc.vector.tensor_copy(
                        v_sb[:, gq * 4:(gq + 1) * 4, :].rearrange("p a b -> p (a b)"),
                        psv)

            # ------------- phase 2+3: attention with fc FIFO interleaved -------------
            with tc.tile_pool(name="out_sb", bufs=3) as out_sb:
                pending = deque()  # fc matmul-chunk / out-dma closures

                def push_fc(sqt):
                    ob = out_sb.tile([128, D], F16, name="ob", tag="ob")

                    def mk(nf):
                        def op():
                            psf = ps8.tile([128, 512], F32, name="psf", tag="pp")
                            for h2 in range(HG):
                                nc.tensor.matmul(psf,
                                                 ctxT[:, h2, sqt * 128:(sqt + 1) * 128],
                                                 fcw_sb[:, h2, nf * 512:(nf + 1) * 512],
                                                 start=(h2 == 0), stop=(h2 == HG - 1))
                            nc.vector.tensor_copy(ob[:, nf * 512:(nf + 1) * 512], psf)
                        return op

                    for nf in range(4):
                        pending.append(mk(nf))
                    pending.append(lambda: nc.sync.dma_start(
                        out=out[sqt * 128:(sqt + 1) * 128, :], in_=ob))

                def scores_op(qc, h, kc):
                    t = kc - 4 * qc
                    pss = ps8.tile([128, 512], F32, name="pss", tag="pp")
                    es = es_pool.tile([128, 512], F16, name="es", tag="es", bufs=6)
                    z = 128 * t if t > 0 else 0  # dead columns on diag tiles
                    if z:
                        nc.vector.memset(es[:, 0:z], 0.0)
                    qs = qt_sb[:, h, qc * 512:(qc + 1) * 512]
                    nc.tensor.matmul(pss[:, z:512], kt_sb[:, kc * 128:(kc + 1) * 128],
                                     qs[:, z:512], start=True, stop=True)
                    nc.scalar.activation(es[:, z:512], pss[:, z:512],
                                         mybir.ActivationFunctionType.Exp,
                                         scale=SCALE)
                    if t >= 0:
                        nc.vector.tensor_tensor(es[:, z:512], es[:, z:512],
                                                mask_sb[:, t, z:512],
                                                op=mybir.AluOpType.mult)
                    return es

                QC_ORDER = (1, 2, 3, 0)
                seq = [(qc, h) for qc in QC_ORDER for h in range(HG)]
                prefetched = None

                for i, (qc, h) in enumerate(seq):
                    nkc = 4 * (qc + 1)  # causal: sk chunks 0..nkc-1
                    npairs = nkc // 2
                    ngroups = nkc // 4
                    psc = ps8.tile([128, 512], F32, name="psc", tag="pp")
                    psd = ps8.tile([1, 512], F32, name="psd", tag="pp")
                    es_tiles = [None] * nkc

                    if prefetched is not None:
                        es_tiles[0], es_tiles[1] = prefetched
                        prefetched = None
                    else:
                        es_tiles[0] = scores_op(qc, h, 0)
                        es_tiles[1] = scores_op(qc, h, 1)

                    # fc-FIFO drain quota: clear the backlog evenly across
                    # the heads of this qc so nothing piles into the tail
                    quota = -(-len(pending) // (HG - h)) if pending else 0
                    drained = 0

                    def dr(n=1):
                        nonlocal drained
                        while drained < quota and n > 0 and pending:
                            pending.popleft()()
                            drained += 1
                            n -= 1

                    def accum_pv(kc):
                        t = kc - 4 * qc
                        z = 128 * t if t > 0 else 0
                        nc.tensor.matmul(psc[:, z:512], v_sb[:, kc, :],
                                         es_tiles[kc][:, z:512],
                                         start=(kc == 0), stop=(kc == nkc - 1))

                    pairs = []
                    groups = []

                    def den_emit(p):
                        esum = es_pool.tile([128, 512], F16, name="esum",
                                            tag="esum", bufs=3)
                        nc.vector.tensor_tensor(esum, es_tiles[2 * p],
                                                es_tiles[2 * p + 1],
                                                op=mybir.AluOpType.add)
                        pairs.append(esum)
                        if len(pairs) == 2:
                            e4 = es_pool.tile([128, 512], F16, name="e4",
                                              tag="e4", bufs=2)
                            nc.gpsimd.tensor_tensor(e4, pairs[0], pairs[1],
                                                    op=mybir.AluOpType.add)
                            del pairs[:]
                            groups.append(e4)

                    def den_mm(g):
                        nc.tensor.matmul(psd, ones_sb, groups[g],
                                         start=(g == 0), stop=(g == ngroups - 1))

                    for p in range(npairs):
                        if p + 1 < npairs:
                            es_tiles[2 * p + 2] = scores_op(qc, h, 2 * p + 2)
                            dr()
                            es_tiles[2 * p + 3] = scores_op(qc, h, 2 * p + 3)
                        accum_pv(2 * p)
                        accum_pv(2 * p + 1)
                        den_emit(p)
                        if p % 2 == 1:
                            g = p // 2
                            if g >= 1:
                                den_mm(g - 1)
                            dr()

                    # prefetch the next head's first score pair: PE work for
                    # ACT/DVE to chew on while this head's denominator lands
                    if i + 1 < len(seq):
                        nqc, nh = seq[i + 1]
                        prefetched = (scores_op(nqc, nh, 0),
                                      scores_op(nqc, nh, 1))

                    den_mm(ngroups - 1)
                    dr(quota - drained)

                    rec = nrm_pool.tile([1, 512], F32, name="rec", tag="rec")
                    nc.vector.reciprocal_approx_fast(rec, psd)
                    rb = nrm_pool.tile([128, 512], F32, name="rb", tag="rb")
                    nc.gpsimd.partition_broadcast(rb, rec)
                    nc.vector.tensor_tensor(ctxT[:, h, qc * 512:(qc + 1) * 512],
                                            psc, rb, op=mybir.AluOpType.mult)

                    if h == HG - 1:
                        for sqt in range(4 * qc, 4 * qc + 4):
                            push_fc(sqt)

                while pending:
                    pending.popleft()()

            nrm_pool.release()
            es_pool.release()
            ps8.release()

    nc.compile()
    return nc


def _get_compiled():
    global _COMPILED
    if _COMPILED is None:
        _COMPILED = _build()
    return _COMPILED


def _prep_inputs(x, w_q, w_kv, fc_w, fc_b, freqs_cos, freqs_sin):
    x = np.asarray(x, dtype=np.float32)
    w_q = np.asarray(w_q, dtype=np.float32)
    w_kv = np.asarray(w_kv, dtype=np.float32)
    fc_w = np.asarray(fc_w, dtype=np.float32)
    freqs_cos = np.asarray(freqs_cos, dtype=np.float32)
    freqs_sin = np.asarray(freqs_sin, dtype=np.float32)

    # rope pair permutation: evens then odds within each head's DK block
    perm = np.concatenate([np.arange(0, DK, 2), np.arange(1, DK, 2)])

    cosT = np.ascontiguousarray(freqs_cos.T).astype(np.float16)  # [64, S]
    sinT = np.ascontiguousarray(freqs_sin.T).astype(np.float16)
    cosT = np.concatenate([cosT, cosT], axis=0)  # duplicate across halves
    sinT = np.concatenate([sinT, sinT], axis=0)

    # masks[i, t, j] = 1 if i <= j - 128*t  (diagonal tiles, t = kc - 4*qc)
    i_idx = np.arange(128)[:, None, None]
    t_idx = np.arange(4)[None, :, None]
    j_idx = np.arange(512)[None, None, :]
    masks = (i_idx <= j_idx - 128 * t_idx).astype(np.float16)
    onesc = np.ones((128, 1), dtype=np.float16)
    iden = np.eye(128, dtype=np.float16)

    in_maps = []
    for c in range(8):
        b, g = divmod(c, 4)
        xT = np.ascontiguousarray(x[b].T).astype(np.float16)
        wq_g = w_q[:, g * HG * DK:(g + 1) * HG * DK].reshape(D, HG, DK)[:, :, perm]
        wq_g = np.ascontiguousarray(wq_g.reshape(D, HG * DK)).astype(np.float16)
        wk_g = np.ascontiguousarray(w_kv[:, g * DK:(g + 1) * DK][:, perm]).astype(np.float16)
        wv_g = np.ascontiguousarray(w_kv[:, HKV * DK + g * DK:HKV * DK + (g + 1) * DK]).astype(np.float16)
        fcw_g = np.ascontiguousarray(fc_w[g * HG * DK:(g + 1) * HG * DK, :]).astype(np.float16)
        in_maps.append({
            "xT": xT, "wq": wq_g, "wk": wk_g, "wv": wv_g, "fcw": fcw_g,
            "cosT": cosT, "sinT": sinT, "masks": masks, "onesc": onesc,
            "iden": iden,
        })
    return in_maps


_WARMED = False


def kernel_run(trace=False, warmup=True, **inputs):
    global _WARMED
    nc = _get_compiled()
    in_maps = _prep_inputs(**inputs)
    if warmup and not _WARMED:
        # first post-compile execution on a cold device is ~15% slower
        # (table loads / HAM state); do a throwaway run
        run_bass_kernel_spmd(nc, in_maps, core_ids=list(range(8)), trace=False)
        _WARMED = True
    res = run_bass_kernel_spmd(nc, in_maps, core_ids=list(range(8)), trace=trace)
    fc_b = np.asarray(inputs["fc_b"], dtype=np.float32)
    out = np.zeros((B, S, D), dtype=np.float32)
    for c in range(8):
        b = c // 4
        out[b] += res.results[c]["out"].astype(np.float32)
    out += fc_b[None, None, :]
    return out, res


def kernel(**inputs):
    out, _ = kernel_run(trace=False, **inputs)
    return out


# revision 10
# speedup vs baseline: 1.6916x; 1.6916x over previous
"""GQA multi-head attention (B=2, S=2048, D=2048, HQ=16, HKV=4, DK=128) with
RoPE + causal softmax + output projection, sharded over 8 NeuronCores as
(batch x kv-head-group): core c handles batch c//4, kv head c%4 (4 query
heads). w_q/w_kv column-sharded, fc row-sharded; partial fc outputs are
summed on the host (the "all-reduce").

Schedule: phase 1 streams xT over two DMA rings (sync/vector) while wq
streams on the scalar ring and misc weights on the gpsimd ring, feeding
kc-outer Q/K/V projection matmuls. Phase 2 runs attention heads in qc
order (1,2,3,0) with a fine-grained FIFO of fc-projection matmul chunks
drained into the attention pipeline's latency bubbles; softmax
denominators use 4-way-summed exp tiles (DVE pair adds + gpsimd combine)
to quarter the PE ones-matmul columns, and PV matmuls skip the dead
columns of diagonal tiles. Partial fc outputs are written in fp16.
"""

import sys
from collections import deque

for _p in ("/opt/trn_rl_repo", "/root/.axon_site", "/root/.axon_site/_ro/trn_rl_repo"):
    if _p not in sys.path:
        sys.path.insert(0, _p)

import numpy as np

import concourse.bass as bass
import concourse.mybir as mybir
import concourse.tile as tile
from concourse import bacc
from concourse.bass_utils import run_bass_kernel_spmd

F32 = mybir.dt.float32
F16 = mybir.dt.float16

B, S, D = 2, 2048, 2048
HKV, NREP, DK = 4, 4, 128
HG = NREP  # query heads per core
KC = D // 128  # contraction chunks
SQC = S // 512  # 512-wide query column chunks
SCALE = float(1.0 / np.sqrt(DK))

_COMPILED = None


def _build():
    nc = bacc.Bacc(None, target_bir_lowering=False, debug=False)

    xT = nc.dram_tensor("xT", [D, S], F16, kind="ExternalInput")
    wq = nc.dram_tensor("wq", [D, HG * DK], F16, kind="ExternalInput")
    wk = nc.dram_tensor("wk", [D, DK], F16, kind="ExternalInput")
    wv = nc.dram_tensor("wv", [D, DK], F16, kind="ExternalInput")
    fcw = nc.dram_tensor("fcw", [HG * DK, D], F16, kind="ExternalInput")
    cosT = nc.dram_tensor("cosT", [128, S], F16, kind="ExternalInput")
    sinT = nc.dram_tensor("sinT", [128, S], F16, kind="ExternalInput")
    masks = nc.dram_tensor("masks", [128, 4, 512], F16, kind="ExternalInput")
    onesc = nc.dram_tensor("onesc", [128, 1], F16, kind="ExternalInput")
    iden = nc.dram_tensor("iden", [128, 128], F16, kind="ExternalInput")
    out = nc.dram_tensor("out", [S, D], F16, kind="ExternalOutput")

    with tile.TileContext(nc) as tc:
        with tc.tile_pool(name="persist", bufs=1) as persist:
            # attention-phase residents
            qt_sb = persist.tile([128, HG, S], F16)  # Q^T, rope'd, per head
            kt_sb = persist.tile([128, S], F16)  # K^T rope'd
            v_sb = persist.tile([128, KC, DK], F16)  # V  [sk, dk] chunks
            ctxT = persist.tile([128, HG, S], F16)  # (softmax @ V)^T per head
            cos_sb = persist.tile([128, S], F16)
            sin_sb = persist.tile([128, S], F16)
            mask_sb = persist.tile([128, 4, 512], F16)
            ones_sb = persist.tile([128, 1], F16)
            iden_sb = persist.tile([128, 128], F16)
            fcw_sb = persist.tile([128, HG, D], F16)

            # pools shared across all phases (no release/realloc barriers)
            ps8 = tc.alloc_tile_pool(name="ps8", bufs=8, space="PSUM")
            es_pool = tc.alloc_tile_pool(name="es_pool", bufs=6)
            nrm_pool = tc.alloc_tile_pool(name="nrm_pool", bufs=3)

            with tc.tile_pool(name="p1sb", bufs=1) as p1sb, \
                 tc.tile_pool(name="p1tmp", bufs=2) as p1tmp:
                xt_sb = p1sb.tile([128, KC, S], F16)
                wq_sb = p1sb.tile([128, KC, HG * DK], F16)
                wk_sb = p1sb.tile([128, KC, DK], F16)
                wv_sb = p1sb.tile([128, KC, DK], F16)
                vt_sb = p1sb.tile([128, S], F16)

                # DMA rings balanced by phase-1 need-time: scalar carries wq
                # (consumed at ~2us/chunk with xT) then the rope/K/V-stage
                # weights; xT is split across the sync and gpsimd rings
                # (vector can't issue DMAs); fcw rides gpsimd last.
                wqr = wq.rearrange("(k p) m -> p k m", p=128)
                xr = xT.rearrange("(k p) s -> p k s", p=128)
                for k in range(KC):
                    nc.scalar.dma_start(out=wq_sb[:, k, :], in_=wqr[:, k, :])
                    eng = nc.sync if k % 2 == 0 else nc.gpsimd
                    eng.dma_start(out=xt_sb[:, k, :], in_=xr[:, k, :])
                nc.scalar.dma_start(out=cos_sb, in_=cosT[:])
                nc.scalar.dma_start(out=sin_sb, in_=sinT[:])
                nc.scalar.dma_start(out=wk_sb, in_=wk.rearrange("(k p) m -> p k m", p=128))
                nc.scalar.dma_start(out=wv_sb, in_=wv.rearrange("(k p) m -> p k m", p=128))
                nc.scalar.dma_start(out=iden_sb, in_=iden[:])
                nc.scalar.dma_start(out=mask_sb, in_=masks[:])
                nc.scalar.dma_start(out=ones_sb, in_=onesc[:])
                nc.gpsimd.dma_start(out=fcw_sb, in_=fcw.rearrange("(h p) n -> p h n", p=128))

                def rope_full(dst, tq):
                    # dst/tq: [128, S] fp16; evens in partitions 0:64, odds 64:128.
                    # cos/sin are duplicated across both halves so every
                    # SBUF*SBUF tensor op has equal input base partitions.
                    pe, po = tq[0:64, :], tq[64:128, :]
                    t1 = p1tmp.tile([64, S], F16, name="t1", tag="t1")
                    t2 = p1tmp.tile([64, S], F16, name="t2", tag="t2")
                    nc.vector.tensor_tensor(t1, pe, cos_sb[0:64, :], op=mybir.AluOpType.mult)
                    nc.vector.tensor_tensor(t2, po, sin_sb[64:128, :], op=mybir.AluOpType.mult)
                    nc.vector.tensor_tensor(dst[0:64, :], t1, t2, op=mybir.AluOpType.subtract)
                    t3 = p1tmp.tile([64, S], F16, name="t3", tag="t1")
                    t4 = p1tmp.tile([64, S], F16, name="t4", tag="t2")
                    nc.vector.tensor_tensor(t3, pe, sin_sb[0:64, :], op=mybir.AluOpType.mult)
                    nc.vector.tensor_tensor(t4, po, cos_sb[64:128, :], op=mybir.AluOpType.mult)
                    nc.vector.tensor_tensor(dst[64:128, :], t3, t4, op=mybir.AluOpType.add)

                # Q^T = wq^T @ xT, kc-outer so PE consumes chunks as they land
                for half in range(2):
                    accs = []
                    for mh in (2 * half, 2 * half + 1):
                        for qc in range(SQC):
                            psq = ps8.tile([128, 512], F32, name="psq", tag="pp")
                            accs.append((mh, qc, psq))
                    for k in range(KC):
                        for mh, qc, psq in accs:
                            nc.tensor.matmul(psq, wq_sb[:, k, mh * 128:(mh + 1) * 128],
                                             xt_sb[:, k, qc * 512:(qc + 1) * 512],
                                             start=(k == 0), stop=(k == KC - 1))
                    tqs = {}
                    for mh in (2 * half, 2 * half + 1):
                        tqs[mh] = p1tmp.tile([128, S], F16, name="tq", tag="tq")
                    for ci, (mh, qc, psq) in enumerate(accs):
                        dst = tqs[mh][:, qc * 512:(qc + 1) * 512]
                        if ci % 2 == 0:
                            nc.scalar.copy(dst, psq)
                        else:
                            nc.vector.tensor_copy(dst, psq)
                    for mh in (2 * half, 2 * half + 1):
                        rope_full(qt_sb[:, mh, :], tqs[mh])

                # K^T = wk^T @ xT
                kaccs = [ps8.tile([128, 512], F32, name="psk", tag="pp")
                         for _ in range(SQC)]
                for k in range(KC):
                    for qc in range(SQC):
                        nc.tensor.matmul(kaccs[qc], wk_sb[:, k, :],
                                         xt_sb[:, k, qc * 512:(qc + 1) * 512],
                                         start=(k == 0), stop=(k == KC - 1))
                tk = p1tmp.tile([128, S], F16, name="tk", tag="tq")
                for qc in range(SQC):
                    dst = tk[:, qc * 512:(qc + 1) * 512]
                    if qc % 2 == 0:
                        nc.scalar.copy(dst, kaccs[qc])
                    else:
                        nc.vector.tensor_copy(dst, kaccs[qc])
                rope_full(kt_sb, tk)

                # V^T = wv^T @ xT (N=512), then PE-transpose to V [sk, dk]
                vaccs = [ps8.tile([128, 512], F32, name="psvt", tag="pp")
                         for _ in range(SQC)]
                for k in range(KC):
                    for sc in range(SQC):
                        nc.tensor.matmul(vaccs[sc], wv_sb[:, k, :],
                                         xt_sb[:, k, sc * 512:(sc + 1) * 512],
                                         start=(k == 0), stop=(k == KC - 1))
                for sc in range(SQC):
                    dst = vt_sb[:, sc * 512:(sc + 1) * 512]
                    if sc % 2 == 0:
                        nc.scalar.copy(dst, vaccs[sc])
                    else:
                        nc.vector.tensor_copy(dst, vaccs[sc])
                for gq in range(4):
                    psv = ps8.tile([128, 512], F16, name="psv", tag="pp")
                    for vt in range(4):
                        skt = gq * 4 + vt
                        nc.tensor.matmul(psv[:, vt * 128:(vt + 1) * 128],
                                         vt_sb[:, skt * 128:(skt + 1) * 128],
                                         iden_sb, is_transpose=True,
                                         start=True, stop=True)
                    nc.vector.tensor_copy(
                        v_sb[:, gq * 4:(gq + 1) * 4, :].rearrange("p a b -> p (a b)"),
                        psv)

            # ------------- phase 2+3: attention with fc FIFO interleaved -------------
            with tc.tile_pool(name="out_sb", bufs=3) as out_sb:
                pending = deque()  # fc matmul-chunk / out-dma closures

                def push_fc(sqt):
                    ob = out_sb.tile([128, D], F16, name="ob", tag="ob")

                    def mk(nf):
                        def op():
                            psf = ps8.tile([128, 512], F32, name="psf", tag="pp")
                            for h2 in range(HG):
                                nc.tensor.matmul(psf,
                                                 ctxT[:, h2, sqt * 128:(sqt + 1) * 128],
                                                 fcw_sb[:, h2, nf * 512:(nf + 1) * 512],
                                                 start=(h2 == 0), stop=(h2 == HG - 1))
                            dst = ob[:, nf * 512:(nf + 1) * 512]
                            if nf % 2 == 0:
                                nc.vector.tensor_copy(dst, psf)
                            else:
                                nc.scalar.copy(dst, psf)
                        return op

                    for nf in range(4):
                        pending.append(mk(nf))
                    pending.append(lambda: nc.sync.dma_start(
                        out=out[sqt * 128:(sqt + 1) * 128, :], in_=ob))

                def scores_op(qc, h, kc):
                    t = kc - 4 * qc
                    pss = ps8.tile([128, 512], F32, name="pss", tag="pp")
                    es = es_pool.tile([128, 512], F16, name="es", tag="es", bufs=6)
                    z = 128 * t if t > 0 else 0  # dead columns on diag tiles
                    qs = qt_sb[:, h, qc * 512:(qc + 1) * 512]
                    nc.tensor.matmul(pss[:, z:512], kt_sb[:, kc * 128:(kc + 1) * 128],
                                     qs[:, z:512], start=True, stop=True)
                    nc.scalar.activation(es[:, z:512], pss[:, z:512],
                                         mybir.ActivationFunctionType.Exp,
                                         scale=SCALE)
                    if t >= 0:
                        # full-width: mask is 0 in the dead columns [0:z], so
                        # this also zeroes the region the exp never wrote
                        # (slot garbage is finite: the ring is primed below)
                        nc.vector.tensor_tensor(es, es, mask_sb[:, t, :],
                                                op=mybir.AluOpType.mult)
                    return es

                # prime the es ring: zero all 6 slots once so the full-width
                # mask multiply never touches uninitialized (possibly NaN)
                # SBUF on the first rotation
                for _ in range(6):
                    es0 = es_pool.tile([128, 512], F16, name="es", tag="es", bufs=6)
                    nc.vector.memset(es0, 0.0)

                QC_ORDER = (1, 2, 3, 0)
                seq = [(qc, h) for qc in QC_ORDER for h in range(HG)]
                prefetched = None

                for i, (qc, h) in enumerate(seq):
                    nkc = 4 * (qc + 1)  # causal: sk chunks 0..nkc-1
                    npairs = nkc // 2
                    ngroups = nkc // 4
                    psc = ps8.tile([128, 512], F32, name="psc", tag="pp")
                    psd = ps8.tile([1, 512], F32, name="psd", tag="pp")
                    es_tiles = [None] * nkc

                    if prefetched is not None:
                        es_tiles[0], es_tiles[1] = prefetched
                        prefetched = None
                    else:
                        es_tiles[0] = scores_op(qc, h, 0)
                        es_tiles[1] = scores_op(qc, h, 1)

                    # fc-FIFO drain quota: clear the backlog evenly across
                    # the heads of this qc so nothing piles into the tail
                    quota = -(-len(pending) // (HG - h)) if pending else 0
                    drained = 0

                    def dr(n=1):
                        nonlocal drained
                        while drained < quota and n > 0 and pending:
                            pending.popleft()()
                            drained += 1
                            n -= 1

                    def accum_pv(kc):
                        t = kc - 4 * qc
                        z = 128 * t if t > 0 else 0
                        nc.tensor.matmul(psc[:, z:512], v_sb[:, kc, :],
                                         es_tiles[kc][:, z:512],
                                         start=(kc == 0), stop=(kc == nkc - 1))

                    pairs = []
                    groups = []

                    def den_emit(p):
                        esum = es_pool.tile([128, 512], F16, name="esum",
                                            tag="esum", bufs=3)
                        nc.vector.tensor_tensor(esum, es_tiles[2 * p],
                                                es_tiles[2 * p + 1],
                                                op=mybir.AluOpType.add)
                        pairs.append(esum)
                        if len(pairs) == 2:
                            # DVE, not gpsimd: mixing op types on gpsimd
                            # forces ~5us microcode lib swaps per switch
                            e4 = es_pool.tile([128, 512], F16, name="e4",
                                              tag="e4", bufs=2)
                            nc.vector.tensor_tensor(e4, pairs[0], pairs[1],
                                                    op=mybir.AluOpType.add)
                            del pairs[:]
                            groups.append(e4)

                    def den_mm(g):
                        nc.tensor.matmul(psd, ones_sb, groups[g],
                                         start=(g == 0), stop=(g == ngroups - 1))

                    for p in range(npairs):
                        if p + 1 < npairs:
                            es_tiles[2 * p + 2] = scores_op(qc, h, 2 * p + 2)
                            dr()
                            es_tiles[2 * p + 3] = scores_op(qc, h, 2 * p + 3)
                        accum_pv(2 * p)
                        accum_pv(2 * p + 1)
                        den_emit(p)
                        if p % 2 == 1:
                            g = p // 2
                            if g >= 1:
                                den_mm(g - 1)
                            dr()

                    # prefetch the next head's first score pair: PE work for
                    # ACT/DVE to chew on while this head's denominator lands
                    if i + 1 < len(seq):
                        nqc, nh = seq[i + 1]
                        prefetched = (scores_op(nqc, nh, 0),
                                      scores_op(nqc, nh, 1))

                    den_mm(ngroups - 1)
                    dr(quota - drained)

                    rec = nrm_pool.tile([1, 512], F32, name="rec", tag="rec")
                    nc.vector.reciprocal_approx_fast(rec, psd)
                    rb = nrm_pool.tile([128, 512], F32, name="rb", tag="rb")
                    nc.gpsimd.partition_broadcast(rb, rec)
                    nc.vector.tensor_tensor(ctxT[:, h, qc * 512:(qc + 1) * 512],
                                            psc, rb, op=mybir.AluOpType.mult)

                    if h == HG - 1:
                        for sqt in range(4 * qc, 4 * qc + 4):
                            push_fc(sqt)

                while pending:
                    pending.popleft()()

            nrm_pool.release()
            es_pool.release()
            ps8.release()

    nc.compile()
    return nc


def _get_compiled():
    global _COMPILED
    if _COMPILED is None:
        _COMPILED = _build()
    return _COMPILED


def _prep_inputs(x, w_q, w_kv, fc_w, fc_b, freqs_cos, freqs_sin):
    x = np.asarray(x, dtype=np.float32)
    w_q = np.asarray(w_q, dtype=np.float32)
    w_kv = np.asarray(w_kv, dtype=np.float32)
    fc_w = np.asarray(fc_w, dtype=np.float32)
    freqs_cos = np.asarray(freqs_cos, dtype=np.float32)
    freqs_sin = np.asarray(freqs_sin, dtype=np.float32)

    # rope pair permutation: evens then odds within each head's DK block
    perm = np.concatenate([np.arange(0, DK, 2), np.arange(1, DK, 2)])

    cosT = np.ascontiguousarray(freqs_cos.T).astype(np.float16)  # [64, S]
    sinT = np.ascontiguousarray(freqs_sin.T).astype(np.float16)
    cosT = np.concatenate([cosT, cosT], axis=0)  # duplicate across halves
    sinT = np.concatenate([sinT, sinT], axis=0)

    # masks[i, t, j] = 1 if i <= j - 128*t  (diagonal tiles, t = kc - 4*qc)
    i_idx = np.arange(128)[:, None, None]
    t_idx = np.arange(4)[None, :, None]
    j_idx = np.arange(512)[None, None, :]
    masks = (i_idx <= j_idx - 128 * t_idx).astype(np.float16)
    onesc = np.ones((128, 1), dtype=np.float16)
    iden = np.eye(128, dtype=np.float16)

    in_maps = []
    for c in range(8):
        b, g = divmod(c, 4)
        xT = np.ascontiguousarray(x[b].T).astype(np.float16)
        wq_g = w_q[:, g * HG * DK:(g + 1) * HG * DK].reshape(D, HG, DK)[:, :, perm]
        wq_g = np.ascontiguousarray(wq_g.reshape(D, HG * DK)).astype(np.float16)
        wk_g = np.ascontiguousarray(w_kv[:, g * DK:(g + 1) * DK][:, perm]).astype(np.float16)
        wv_g = np.ascontiguousarray(w_kv[:, HKV * DK + g * DK:HKV * DK + (g + 1) * DK]).astype(np.float16)
        fcw_g = np.ascontiguousarray(fc_w[g * HG * DK:(g + 1) * HG * DK, :]).astype(np.float16)
        in_maps.append({
            "xT": xT, "wq": wq_g, "wk": wk_g, "wv": wv_g, "fcw": fcw_g,
            "cosT": cosT, "sinT": sinT, "masks": masks, "onesc": onesc,
            "iden": iden,
        })
    return in_maps


_WARMED = False


def kernel_run(trace=False, warmup=True, **inputs):
    global _WARMED
    nc = _get_compiled()
    in_maps = _prep_inputs(**inputs)
    if warmup and not _WARMED:
        # first post-compile execution on a cold device is ~15% slower
        # (table loads / HAM state); do a throwaway run
        run_bass_kernel_spmd(nc, in_maps, core_ids=list(range(8)), trace=False)
        _WARMED = True
    res = run_bass_kernel_spmd(nc, in_maps, core_ids=list(range(8)), trace=trace)
    fc_b = np.asarray(inputs["fc_b"], dtype=np.float32)
    out = np.zeros((B, S, D), dtype=np.float32)
    for c in range(8):
        b = c // 4
        out[b] += res.results[c]["out"].astype(np.float32)
    out += fc_b[None, None, :]
    return out, res


def kernel(**inputs):
    out, _ = kernel_run(trace=False, **inputs)
    return out


# revision 19
# speedup vs baseline: 1.7260x; 1.0203x over previous
"""GQA multi-head attention (B=2, S=2048, D=2048, HQ=16, HKV=4, DK=128) with
RoPE + causal softmax + output projection, sharded over 8 NeuronCores as
(batch x kv-head-group): core c handles batch c//4, kv head c%4 (4 query
heads). w_q/w_kv column-sharded, fc row-sharded; partial fc outputs are
summed on the host (the "all-reduce").

Schedule: phase 1 streams xT over two DMA rings (sync/vector) while wq
streams on the scalar ring and misc weights on the gpsimd ring, feeding
kc-outer Q/K/V projection matmuls. Phase 2 runs attention heads in qc
order (1,2,3,0) with a fine-grained FIFO of fc-projection matmul chunks
drained into the attention pipeline's latency bubbles; softmax
denominators use 4-way-summed exp tiles (DVE pair adds + gpsimd combine)
to quarter the PE ones-matmul columns, and PV matmuls skip the dead
columns of diagonal tiles. Partial fc outputs are written in fp16.
"""

import sys
from collections import deque

for _p in ("/opt/trn_rl_repo", "/root/.axon_site", "/root/.axon_site/_ro/trn_rl_repo"):
    if _p not in sys.path:
        sys.path.insert(0, _p)

import numpy as np

import concourse.bass as bass
import concourse.mybir as mybir
import concourse.tile as tile
from concourse import bacc
from concourse.bass_utils import run_bass_kernel_spmd

F32 = mybir.dt.float32
F16 = mybir.dt.float16

B, S, D = 2, 2048, 2048
HKV, NREP, DK = 4, 4, 128
HG = NREP  # query heads per core
KC = D // 128  # contraction chunks
SQC = S // 512  # 512-wide query column chunks
SCALE = float(1.0 / np.sqrt(DK))

_COMPILED = None


def _build():
    nc = bacc.Bacc(None, target_bir_lowering=False, debug=False)

    xT = nc.dram_tensor("xT", [D, S], F16, kind="ExternalInput")
    # wq packed host-side as [128, 4, 2048]: group j row = chunks 4j..4j+3
    # concatenated, giving 4KB contiguous DMA rows (1KB rows starve the ring)
    wq = nc.dram_tensor("wq", [128, 4, 4 * 512], F16, kind="ExternalInput")
    wk = nc.dram_tensor("wk", [128, KC * DK], F16, kind="ExternalInput")
    wv = nc.dram_tensor("wv", [128, KC * DK], F16, kind="ExternalInput")
    fcw = nc.dram_tensor("fcw", [HG * DK, D], F16, kind="ExternalInput")
    cosT = nc.dram_tensor("cosT", [128, S], F16, kind="ExternalInput")
    sinT = nc.dram_tensor("sinT", [128, S], F16, kind="ExternalInput")
    masks = nc.dram_tensor("masks", [128, 4, 512], F16, kind="ExternalInput")
    onesc = nc.dram_tensor("onesc", [128, 1], F16, kind="ExternalInput")
    iden = nc.dram_tensor("iden", [128, 128], F16, kind="ExternalInput")
    out = nc.dram_tensor("out", [S, D], F16, kind="ExternalOutput")

    with tile.TileContext(nc) as tc:
        with tc.tile_pool(name="persist", bufs=1) as persist:
            # attention-phase residents
            qt_sb = persist.tile([128, HG, S], F16)  # Q^T, rope'd, per head
            kt_sb = persist.tile([128, S], F16)  # K^T rope'd
            v_sb = persist.tile([128, KC, DK], F16)  # V  [sk, dk] chunks
            ctxT = persist.tile([128, HG, S], F16)  # (softmax @ V)^T per head
            cos_sb = persist.tile([128, S], F16)
            sin_sb = persist.tile([128, S], F16)
            mask_sb = persist.tile([128, 4, 512], F16)
            ones_sb = persist.tile([128, 1], F16)
            iden_sb = persist.tile([128, 128], F16)
            fcw_sb = persist.tile([128, HG, D], F16)

            # pools shared across all phases (no release/realloc barriers)
            ps8 = tc.alloc_tile_pool(name="ps8", bufs=8, space="PSUM")
            es_pool = tc.alloc_tile_pool(name="es_pool", bufs=6)
            nrm_pool = tc.alloc_tile_pool(name="nrm_pool", bufs=2)

            with tc.tile_pool(name="p1sb", bufs=1) as p1sb, \
                 tc.tile_pool(name="p1tmp", bufs=2) as p1tmp:
                xt_sb = p1sb.tile([128, KC, S], F16)
                wq_sb = p1sb.tile([128, KC, HG * DK], F16)
                wk_sb = p1sb.tile([128, KC, DK], F16)
                wv_sb = p1sb.tile([128, KC, DK], F16)
                vt_sb = p1sb.tile([128, S], F16)

                # DMA rings balanced by phase-1 need-time and measured ring
                # rates (gpsimd > sync; byte-rate scales with row size):
                # scalar carries wq groups + rope/K/V weights, xT is split
                # 6/10 across sync/gpsimd, fcw rides gpsimd last.
                xr = xT.rearrange("(k p) s -> p k s", p=128)
                nc.sync.dma_start(out=xt_sb[:, 0, 0:1024], in_=xr[:, 0, 0:1024])
                nc.sync.dma_start(out=xt_sb[:, 0, 1024:2048], in_=xr[:, 0, 1024:2048])
                for k in (2, 4, 6, 8, 10):
                    nc.sync.dma_start(out=xt_sb[:, k, :], in_=xr[:, k, :])
                for k in (1, 3, 5, 7, 9, 11, 12, 13, 14, 15):
                    nc.gpsimd.dma_start(out=xt_sb[:, k, :], in_=xr[:, k, :])
                for j in range(4):
                    nc.scalar.dma_start(
                        out=wq_sb[:, 4 * j:4 * (j + 1), :].rearrange("p a b -> p (a b)"),
                        in_=wq[:, j, :])
                nc.scalar.dma_start(out=cos_sb, in_=cosT[:])
                nc.scalar.dma_start(out=sin_sb, in_=sinT[:])
                nc.scalar.dma_start(out=wk_sb.rearrange("p a b -> p (a b)"), in_=wk[:])
                nc.scalar.dma_start(out=wv_sb.rearrange("p a b -> p (a b)"), in_=wv[:])
                nc.sync.dma_start(out=iden_sb, in_=iden[:])
                nc.sync.dma_start(out=mask_sb, in_=masks[:])
                nc.sync.dma_start(out=ones_sb, in_=onesc[:])
                nc.gpsimd.dma_start(out=fcw_sb, in_=fcw.rearrange("(h p) n -> p h n", p=128))

                def rope_full(dst, tq):
                    # dst/tq: [128, S] fp16; evens in partitions 0:64, odds 64:128.
                    # cos/sin are duplicated across both halves so every
                    # SBUF*SBUF tensor op has equal input base partitions.
                    pe, po = tq[0:64, :], tq[64:128, :]
                    t1 = p1tmp.tile([64, S], F16, name="t1", tag="t1")
                    t2 = p1tmp.tile([64, S], F16, name="t2", tag="t2")
                    nc.vector.tensor_tensor(t1, pe, cos_sb[0:64, :], op=mybir.AluOpType.mult)
                    nc.vector.tensor_tensor(t2, po, sin_sb[64:128, :], op=mybir.AluOpType.mult)
                    nc.vector.tensor_tensor(dst[0:64, :], t1, t2, op=mybir.AluOpType.subtract)
                    t3 = p1tmp.tile([64, S], F16, name="t3", tag="t1")
                    t4 = p1tmp.tile([64, S], F16, name="t4", tag="t2")
                    nc.vector.tensor_tensor(t3, pe, sin_sb[0:64, :], op=mybir.AluOpType.mult)
                    nc.vector.tensor_tensor(t4, po, cos_sb[64:128, :], op=mybir.AluOpType.mult)
                    nc.vector.tensor_tensor(dst[64:128, :], t3, t4, op=mybir.AluOpType.add)

                # Q^T = wq^T @ xT, kc-outer so PE consumes chunks as they land
                for half in range(2):
                    accs = []
                    for mh in (2 * half, 2 * half + 1):
                        for qc in range(SQC):
                            psq = ps8.tile([128, 512], F32, name="psq", tag="pp")
                            accs.append((mh, qc, psq))
                    for k in range(KC):
                        for mh, qc, psq in accs:
                            nc.tensor.matmul(psq, wq_sb[:, k, mh * 128:(mh + 1) * 128],
                                             xt_sb[:, k, qc * 512:(qc + 1) * 512],
                                             start=(k == 0), stop=(k == KC - 1))
                    tqs = {}
                    for mh in (2 * half, 2 * half + 1):
                        tqs[mh] = p1tmp.tile([128, S], F16, name="tq", tag="tq")
                    for ci, (mh, qc, psq) in enumerate(accs):
                        dst = tqs[mh][:, qc * 512:(qc + 1) * 512]
                        if ci % 2 == 0:
                            nc.scalar.copy(dst, psq)
                        else:
                            nc.vector.tensor_copy(dst, psq)
                    for mh in (2 * half, 2 * half + 1):
                        rope_full(qt_sb[:, mh, :], tqs[mh])

                # K^T = wk^T @ xT
                kaccs = [ps8.tile([128, 512], F32, name="psk", tag="pp")
                         for _ in range(SQC)]
                for k in range(KC):
                    for qc in range(SQC):
                        nc.tensor.matmul(kaccs[qc], wk_sb[:, k, :],
                                         xt_sb[:, k, qc * 512:(qc + 1) * 512],
                                         start=(k == 0), stop=(k == KC - 1))
                tk = p1tmp.tile([128, S], F16, name="tk", tag="tq")
                for qc in range(SQC):
                    dst = tk[:, qc * 512:(qc + 1) * 512]
                    if qc % 2 == 0:
                        nc.scalar.copy(dst, kaccs[qc])
                    else:
                        nc.vector.tensor_copy(dst, kaccs[qc])
                rope_full(kt_sb, tk)

                # V^T = wv^T @ xT (N=512), then PE-transpose to V [sk, dk]
                vaccs = [ps8.tile([128, 512], F32, name="psvt", tag="pp")
                         for _ in range(SQC)]
                for k in range(KC):
                    for sc in range(SQC):
                        nc.tensor.matmul(vaccs[sc], wv_sb[:, k, :],
                                         xt_sb[:, k, sc * 512:(sc + 1) * 512],
                                         start=(k == 0), stop=(k == KC - 1))
                for sc in range(SQC):
                    dst = vt_sb[:, sc * 512:(sc + 1) * 512]
                    if sc % 2 == 0:
                        nc.scalar.copy(dst, vaccs[sc])
                    else:
                        nc.vector.tensor_copy(dst, vaccs[sc])
                for gq in range(4):
                    psv = ps8.tile([128, 512], F16, name="psv", tag="pp")
                    for vt in range(4):
                        skt = gq * 4 + vt
                        nc.tensor.matmul(psv[:, vt * 128:(vt + 1) * 128],
                                         vt_sb[:, skt * 128:(skt + 1) * 128],
                                         iden_sb, is_transpose=True,
                                         start=True, stop=True)
                    nc.vector.tensor_copy(
                        v_sb[:, gq * 4:(gq + 1) * 4, :].rearrange("p a b -> p (a b)"),
                        psv)

            # ------------- phase 2+3: attention with fc FIFO interleaved -------------
            with tc.tile_pool(name="out_sb", bufs=3) as out_sb:
                pending = deque()  # fc matmul-chunk / out-dma closures

                def push_fc(sqt):
                    ob = out_sb.tile([128, D], F16, name="ob", tag="ob")

                    def mk(nf):
                        def op():
                            psf = ps8.tile([128, 512], F32, name="psf", tag="pp")
                            for h2 in range(HG):
                                nc.tensor.matmul(psf,
                                                 ctxT[:, h2, sqt * 128:(sqt + 1) * 128],
                                                 fcw_sb[:, h2, nf * 512:(nf + 1) * 512],
                                                 start=(h2 == 0), stop=(h2 == HG - 1))
                            dst = ob[:, nf * 512:(nf + 1) * 512]
                            if nf % 2 == 0:
                                nc.vector.tensor_copy(dst, psf)
                            else:
                                nc.scalar.copy(dst, psf)
                        return op

                    for nf in range(4):
                        pending.append(mk(nf))
                        if nf % 2 == 1:
                            half = nf // 2
                            pending.append(lambda half=half, ob=ob: nc.sync.dma_start(
                                out=out[sqt * 128:(sqt + 1) * 128,
                                        half * 1024:(half + 1) * 1024],
                                in_=ob[:, half * 1024:(half + 1) * 1024]))

                def scores_op(qc, h, kc):
                    t = kc - 4 * qc
                    pss = ps8.tile([128, 512], F32, name="pss", tag="pp")
                    es = es_pool.tile([128, 512], F16, name="es", tag="es", bufs=8)
                    z = 128 * t if t > 0 else 0  # dead columns on diag tiles
                    qs = qt_sb[:, h, qc * 512:(qc + 1) * 512]
                    nc.tensor.matmul(pss[:, z:512], kt_sb[:, kc * 128:(kc + 1) * 128],
                                     qs[:, z:512], start=True, stop=True)
                    nc.scalar.activation(es[:, z:512], pss[:, z:512],
                                         mybir.ActivationFunctionType.Exp,
                                         scale=SCALE)
                    if t >= 0:
                        # full-width: mask is 0 in the dead columns [0:z], so
                        # this also zeroes the region the exp never wrote
                        # (slot garbage is finite: the ring is primed below)
                        nc.vector.tensor_tensor(es, es, mask_sb[:, t, :],
                                                op=mybir.AluOpType.mult)
                    return es

                # prime the es ring: zero all 6 slots once so the full-width
                # mask multiply never touches uninitialized (possibly NaN)
                # SBUF on the first rotation
                for _ in range(8):
                    es0 = es_pool.tile([128, 512], F16, name="es", tag="es", bufs=8)
                    nc.vector.memset(es0, 0.0)

                QC_ORDER = (1, 2, 3, 0)
                seq = [(qc, h) for qc in QC_ORDER for h in range(HG)]
                prefetched = None

                for i, (qc, h) in enumerate(seq):
                    nkc = 4 * (qc + 1)  # causal: sk chunks 0..nkc-1
                    npairs = nkc // 2
                    ngroups = nkc // 4
                    psc = ps8.tile([128, 512], F32, name="psc", tag="pp")
                    psd = ps8.tile([1, 512], F32, name="psd", tag="pp")
                    es_tiles = [None] * nkc

                    issued = 0
                    if prefetched is not None:
                        for es in prefetched:
                            es_tiles[issued] = es
                            issued += 1
                        prefetched = None

                    def ensure_scores(upto):
                        nonlocal issued
                        while issued < min(upto, nkc):
                            es_tiles[issued] = scores_op(qc, h, issued)
                            issued += 1

                    ensure_scores(4 if i == 0 else 2)

                    # fc-FIFO drain quota: clear the backlog evenly across
                    # the heads of this qc so nothing piles into the tail
                    quota = -(-len(pending) // (HG - h)) if pending else 0
                    drained = 0

                    def dr(n=1):
                        nonlocal drained
                        while drained < quota and n > 0 and pending:
                            pending.popleft()()
                            drained += 1
                            n -= 1

                    def accum_pv(kc):
                        t = kc - 4 * qc
                        z = 128 * t if t > 0 else 0
                        nc.tensor.matmul(psc[:, z:512], v_sb[:, kc, :],
                                         es_tiles[kc][:, z:512],
                                         start=(kc == 0), stop=(kc == nkc - 1))

                    pairs = []
                    groups = []

                    def den_emit(p):
                        esum = es_pool.tile([128, 512], F16, name="esum",
                                            tag="esum", bufs=3)
                        nc.vector.tensor_tensor(esum, es_tiles[2 * p],
                                                es_tiles[2 * p + 1],
                                                op=mybir.AluOpType.add)
                        pairs.append(esum)
                        if len(pairs) == 2:
                            # DVE, not gpsimd: mixing op types on gpsimd
                            # forces ~5us microcode lib swaps per switch
                            e4 = es_pool.tile([128, 512], F16, name="e4",
                                              tag="e4", bufs=2)
                            nc.vector.tensor_tensor(e4, pairs[0], pairs[1],
                                                    op=mybir.AluOpType.add)
                            del pairs[:]
                            groups.append(e4)

                    def den_mm(g):
                        nc.tensor.matmul(psd, ones_sb, groups[g],
                                         start=(g == 0), stop=(g == ngroups - 1))

                    for p in range(npairs):
                        if p + 1 < npairs:
                            ensure_scores(2 * p + 3)
                            dr()
                            ensure_scores(2 * p + 4)
                        accum_pv(2 * p)
                        accum_pv(2 * p + 1)
                        den_emit(p)
                        if p % 2 == 1:
                            g = p // 2
                            if g >= 1:
                                den_mm(g - 1)
                            dr()

                    # prefetch the next head's first scores: PE work for
                    # ACT/DVE to chew on while this head's denominator lands
                    # (deeper for short qc=0 heads, which have little other
                    # pipeline to hide the den/normalize chain behind)
                    if i + 1 < len(seq):
                        nqc, nh = seq[i + 1]
                        depth = 4 if nqc == 0 else 2
                        prefetched = [scores_op(nqc, nh, kc) for kc in range(depth)]

                    dr(quota - drained)
                    den_mm(ngroups - 1)

                    rec = nrm_pool.tile([1, 512], F32, name="rec", tag="rec")
                    nc.vector.reciprocal_approx_fast(rec, psd)
                    rb = nrm_pool.tile([128, 512], F32, name="rb", tag="rb")
                    nc.gpsimd.partition_broadcast(rb, rec)
                    nc.vector.tensor_tensor(ctxT[:, h, qc * 512:(qc + 1) * 512],
                                            psc, rb, op=mybir.AluOpType.mult)

                    if h == HG - 1:
                        for sqt in range(4 * qc, 4 * qc + 4):
                            push_fc(sqt)

                while pending:
                    pending.popleft()()

            nrm_pool.release()
            es_pool.release()
            ps8.release()

    nc.compile()
    return nc


def _get_compiled():
    global _COMPILED
    if _COMPILED is None:
        _COMPILED = _build()
    return _COMPILED


def _prep_inputs(x, w_q, w_kv, fc_w, fc_b, freqs_cos, freqs_sin):
    x = np.asarray(x, dtype=np.float32)
    w_q = np.asarray(w_q, dtype=np.float32)
    w_kv = np.asarray(w_kv, dtype=np.float32)
    fc_w = np.asarray(fc_w, dtype=np.float32)
    freqs_cos = np.asarray(freqs_cos, dtype=np.float32)
    freqs_sin = np.asarray(freqs_sin, dtype=np.float32)

    # rope pair permutation: evens then odds within each head's DK block
    perm = np.concatenate([np.arange(0, DK, 2), np.arange(1, DK, 2)])

    cosT = np.ascontiguousarray(freqs_cos.T).astype(np.float16)  # [64, S]
    sinT = np.ascontiguousarray(freqs_sin.T).astype(np.float16)
    cosT = np.concatenate([cosT, cosT], axis=0)  # duplicate across halves
    sinT = np.concatenate([sinT, sinT], axis=0)

    # masks[i, t, j] = 1 if i <= j - 128*t  (diagonal tiles, t = kc - 4*qc)
    i_idx = np.arange(128)[:, None, None]
    t_idx = np.arange(4)[None, :, None]
    j_idx = np.arange(512)[None, None, :]
    masks = (i_idx <= j_idx - 128 * t_idx).astype(np.float16)
    onesc = np.ones((128, 1), dtype=np.float16)
    iden = np.eye(128, dtype=np.float16)

    in_maps = []
    for c in range(8):
        b, g = divmod(c, 4)
        xT = np.ascontiguousarray(x[b].T).astype(np.float16)
        wq_g = w_q[:, g * HG * DK:(g + 1) * HG * DK].reshape(D, HG, DK)[:, :, perm]
        wq_g = wq_g.reshape(D, HG * DK)
        # pack [2048, 512] -> [128, 4 groups, 4*512]: chunk k rows 128k..128k+127
        # land on partition p = row % 128, 4 chunks concatenated per group row
        wq_g = np.ascontiguousarray(
            wq_g.reshape(4, 4, 128, HG * DK).transpose(2, 0, 1, 3)
                .reshape(128, 4, 4 * HG * DK)).astype(np.float16)
        wk_g = w_kv[:, g * DK:(g + 1) * DK][:, perm]
        wk_g = np.ascontiguousarray(
            wk_g.reshape(KC, 128, DK).transpose(1, 0, 2).reshape(128, KC * DK)
        ).astype(np.float16)
        wv_g = w_kv[:, HKV * DK + g * DK:HKV * DK + (g + 1) * DK]
        wv_g = np.ascontiguousarray(
            wv_g.reshape(KC, 128, DK).transpose(1, 0, 2).reshape(128, KC * DK)
        ).astype(np.float16)
        fcw_g = np.ascontiguousarray(fc_w[g * HG * DK:(g + 1) * HG * DK, :]).astype(np.float16)
        in_maps.append({
            "xT": xT, "wq": wq_g, "wk": wk_g, "wv": wv_g, "fcw": fcw_g,
            "cosT": cosT, "sinT": sinT, "masks": masks, "onesc": onesc,
            "iden": iden,
        })
    return in_maps


_WARMED = False


def kernel_run(trace=False, warmup=True, **inputs):
    global _WARMED
    nc = _get_compiled()
    in_maps = _prep_inputs(**inputs)
    if warmup and not _WARMED:
        # first post-compile execution on a cold device is ~15% slower
        # (table loads / HAM state); do a throwaway run
        run_bass_kernel_spmd(nc, in_maps, core_ids=list(range(8)), trace=False)
        _WARMED = True
    res = run_bass_kernel_spmd(nc, in_maps, core_ids=list(range(8)), trace=trace)
    fc_b = np.asarray(inputs["fc_b"], dtype=np.float32)
    out = np.zeros((B, S, D), dtype=np.float32)
    for c in range(8):
        b = c // 4
        out[b] += res.results[c]["out"].astype(np.float32)
    out += fc_b[None, None, :]
    return out, res


def kernel(**inputs):
    out, _ = kernel_run(trace=False, **inputs)
    return out


# revision 25
# speedup vs baseline: 1.7438x; 1.0103x over previous
"""GQA multi-head attention (B=2, S=2048, D=2048, HQ=16, HKV=4, DK=128) with
RoPE + causal softmax + output projection, sharded over 8 NeuronCores as
(batch x kv-head-group): core c handles batch c//4, kv head c%4 (4 query
heads). w_q/w_kv column-sharded, fc row-sharded; partial fc outputs are
summed on the host (the "all-reduce").

Schedule: phase 1 streams xT over two DMA rings (sync/vector) while wq
streams on the scalar ring and misc weights on the gpsimd ring, feeding
kc-outer Q/K/V projection matmuls. Phase 2 runs attention heads in qc
order (1,2,3,0) with a fine-grained FIFO of fc-projection matmul chunks
drained into the attention pipeline's latency bubbles; softmax
denominators use 4-way-summed exp tiles (DVE pair adds + gpsimd combine)
to quarter the PE ones-matmul columns, and PV matmuls skip the dead
columns of diagonal tiles. Partial fc outputs are written in fp16.
"""

import sys
from collections import deque

for _p in ("/opt/trn_rl_repo", "/root/.axon_site", "/root/.axon_site/_ro/trn_rl_repo"):
    if _p not in sys.path:
        sys.path.insert(0, _p)

import numpy as np

import concourse.bass as bass
import concourse.mybir as mybir
import concourse.tile as tile
from concourse import bacc
from concourse.bass_utils import run_bass_kernel_spmd

F32 = mybir.dt.float32
F16 = mybir.dt.float16

B, S, D = 2, 2048, 2048
HKV, NREP, DK = 4, 4, 128
HG = NREP  # query heads per core
KC = D // 128  # contraction chunks
SQC = S // 512  # 512-wide query column chunks
SCALE = float(1.0 / np.sqrt(DK))

_COMPILED = None


def _build():
    nc = bacc.Bacc(None, target_bir_lowering=False, debug=False)

    xT = nc.dram_tensor("xT", [D, S], F16, kind="ExternalInput")
    # wq packed host-side as [128, 4, 2048]: group j row = chunks 4j..4j+3
    # concatenated, giving 4KB contiguous DMA rows (1KB rows starve the ring)
    wq = nc.dram_tensor("wq", [128, 4, 4 * 512], F16, kind="ExternalInput")
    wk = nc.dram_tensor("wk", [128, KC * DK], F16, kind="ExternalInput")
    wv = nc.dram_tensor("wv", [128, KC * DK], F16, kind="ExternalInput")
    fcw = nc.dram_tensor("fcw", [HG * DK, D], F16, kind="ExternalInput")
    cosT = nc.dram_tensor("cosT", [128, S], F16, kind="ExternalInput")
    sinT = nc.dram_tensor("sinT", [128, S], F16, kind="ExternalInput")
    masks = nc.dram_tensor("masks", [128, 4, 512], F16, kind="ExternalInput")
    onesc = nc.dram_tensor("onesc", [128, 1], F16, kind="ExternalInput")
    iden = nc.dram_tensor("iden", [128, 128], F16, kind="ExternalInput")
    out = nc.dram_tensor("out", [S, D], F16, kind="ExternalOutput")

    with tile.TileContext(nc) as tc:
        with tc.tile_pool(name="persist", bufs=1) as persist:
            # attention-phase residents
            qt_sb = persist.tile([128, HG, S], F16)  # Q^T, rope'd, per head
            kt_sb = persist.tile([128, S], F16)  # K^T rope'd
            v_sb = persist.tile([128, KC, DK], F16)  # V  [sk, dk] chunks
            ctxT = persist.tile([128, HG, S], F16)  # (softmax @ V)^T per head
            cos_sb = persist.tile([128, S], F16)
            sin_sb = persist.tile([128, S], F16)
            mask_sb = persist.tile([128, 4, 512], F16)
            ones_sb = persist.tile([128, 1], F16)
            iden_sb = persist.tile([128, 128], F16)
            fcw_sb = persist.tile([128, HG, D], F16)
            # persist (not phase-1-scoped): consumed by the Qh1/V-transpose
            # filler ops that run interleaved into the attention phase
            xt_sb = persist.tile([128, KC, S], F16)
            wq_sb = persist.tile([128, KC, HG * DK], F16)
            vt_sb = persist.tile([128, S], F16)

            # pools shared across all phases (no release/realloc barriers)
            ps8 = tc.alloc_tile_pool(name="ps8", bufs=8, space="PSUM")
            es_pool = tc.alloc_tile_pool(name="es_pool", bufs=6)
            nrm_pool = tc.alloc_tile_pool(name="nrm_pool", bufs=2)

            with tc.tile_pool(name="p1sb", bufs=1) as p1sb, \
                 tc.tile_pool(name="p1tmp", bufs=2) as p1tmp:
                wk_sb = p1sb.tile([128, KC, DK], F16)
                wv_sb = p1sb.tile([128, KC, DK], F16)

                # DMA rings balanced by phase-1 need-time and measured ring
                # rates (gpsimd > sync; byte-rate scales with row size):
                # scalar carries wq groups + K/V weights, xT is split
                # 6/10 across sync/gpsimd, fcw rides gpsimd last.
                xr = xT.rearrange("(k p) s -> p k s", p=128)
                nc.sync.dma_start(out=xt_sb[:, 0, 0:512], in_=xr[:, 0, 0:512])
                nc.sync.dma_start(out=xt_sb[:, 0, 512:2048], in_=xr[:, 0, 512:2048])
                for k in (2, 4, 6, 8, 10):
                    nc.sync.dma_start(out=xt_sb[:, k, :], in_=xr[:, k, :])
                for k in (1, 3, 5, 7, 9, 11, 12, 13, 14, 15):
                    nc.gpsimd.dma_start(out=xt_sb[:, k, :], in_=xr[:, k, :])
                # wq chunk 0 first and solo so the first matmul starts ASAP
                nc.scalar.dma_start(out=wq_sb[:, 0, :], in_=wq[:, 0, 0:512])
                nc.scalar.dma_start(
                    out=wq_sb[:, 1:4, :].rearrange("p a b -> p (a b)"),
                    in_=wq[:, 0, 512:2048])
                nc.scalar.dma_start(
                    out=wq_sb[:, 4:8, :].rearrange("p a b -> p (a b)"),
                    in_=wq[:, 1, :])
                nc.scalar.dma_start(out=wk_sb.rearrange("p a b -> p (a b)"), in_=wk[:])
                nc.scalar.dma_start(out=wv_sb.rearrange("p a b -> p (a b)"), in_=wv[:])
                for j in (2, 3):
                    nc.scalar.dma_start(
                        out=wq_sb[:, 4 * j:4 * (j + 1), :].rearrange("p a b -> p (a b)"),
                        in_=wq[:, j, :])
                nc.scalar.dma_start(out=cos_sb, in_=cosT[:])
                nc.scalar.dma_start(out=sin_sb, in_=sinT[:])
                nc.sync.dma_start(out=iden_sb, in_=iden[:])
                nc.sync.dma_start(out=mask_sb, in_=masks[:])
                nc.sync.dma_start(out=ones_sb, in_=onesc[:])
                nc.gpsimd.dma_start(out=fcw_sb, in_=fcw.rearrange("(h p) n -> p h n", p=128))

                def rope_full(dst, tq):
                    # dst/tq: [128, S] fp16; evens in partitions 0:64, odds 64:128.
                    # cos/sin are duplicated across both halves so every
                    # SBUF*SBUF tensor op has equal input base partitions.
                    pe, po = tq[0:64, :], tq[64:128, :]
                    t1 = p1tmp.tile([64, S], F16, name="t1", tag="t1", bufs=1)
                    t2 = p1tmp.tile([64, S], F16, name="t2", tag="t2", bufs=1)
                    nc.vector.tensor_tensor(t1, pe, cos_sb[0:64, :], op=mybir.AluOpType.mult)
                    nc.vector.tensor_tensor(t2, po, sin_sb[64:128, :], op=mybir.AluOpType.mult)
                    nc.vector.tensor_tensor(dst[0:64, :], t1, t2, op=mybir.AluOpType.subtract)
                    t3 = p1tmp.tile([64, S], F16, name="t3", tag="t1", bufs=1)
                    t4 = p1tmp.tile([64, S], F16, name="t4", tag="t2", bufs=1)
                    nc.vector.tensor_tensor(t3, pe, sin_sb[0:64, :], op=mybir.AluOpType.mult)
                    nc.vector.tensor_tensor(t4, po, cos_sb[64:128, :], op=mybir.AluOpType.mult)
                    nc.vector.tensor_tensor(dst[64:128, :], t3, t4, op=mybir.AluOpType.add)

                # Q^T = wq^T @ xT for heads 0,1 only — heads 2,3 are deferred
                # into the attention phase as PE filler ops. kc-outer so PE
                # consumes chunks as they land.
                accs = []
                for mh in (0, 1):
                    for qc in range(SQC):
                        psq = ps8.tile([128, 512], F32, name="psq", tag="pp")
                        accs.append((mh, qc, psq))
                for k in range(KC):
                    for mh, qc, psq in accs:
                        nc.tensor.matmul(psq, wq_sb[:, k, mh * 128:(mh + 1) * 128],
                                         xt_sb[:, k, qc * 512:(qc + 1) * 512],
                                         start=(k == 0), stop=(k == KC - 1))
                tqs = {}
                for mh in (0, 1):
                    tqs[mh] = p1tmp.tile([128, S], F16, name="tq", tag="tq")
                for ci, (mh, qc, psq) in enumerate(accs):
                    dst = tqs[mh][:, qc * 512:(qc + 1) * 512]
                    if ci % 2 == 0:
                        nc.scalar.copy(dst, psq)
                    else:
                        nc.vector.tensor_copy(dst, psq)
                for mh in (0, 1):
                    rope_full(qt_sb[:, mh, :], tqs[mh])

                # K^T = wk^T @ xT
                kaccs = [ps8.tile([128, 512], F32, name="psk", tag="pp")
                         for _ in range(SQC)]
                for k in range(KC):
                    for qc in range(SQC):
                        nc.tensor.matmul(kaccs[qc], wk_sb[:, k, :],
                                         xt_sb[:, k, qc * 512:(qc + 1) * 512],
                                         start=(k == 0), stop=(k == KC - 1))
                tk = p1tmp.tile([128, S], F16, name="tk", tag="tq")
                for qc in range(SQC):
                    dst = tk[:, qc * 512:(qc + 1) * 512]
                    if qc % 2 == 0:
                        nc.scalar.copy(dst, kaccs[qc])
                    else:
                        nc.vector.tensor_copy(dst, kaccs[qc])
                rope_full(kt_sb, tk)

                # V^T = wv^T @ xT (N=512), then PE-transpose to V [sk, dk]
                vaccs = [ps8.tile([128, 512], F32, name="psvt", tag="pp")
                         for _ in range(SQC)]
                for k in range(KC):
                    for sc in range(SQC):
                        nc.tensor.matmul(vaccs[sc], wv_sb[:, k, :],
                                         xt_sb[:, k, sc * 512:(sc + 1) * 512],
                                         start=(k == 0), stop=(k == KC - 1))
                for sc in range(SQC):
                    dst = vt_sb[:, sc * 512:(sc + 1) * 512]
                    if sc % 2 == 0:
                        nc.scalar.copy(dst, vaccs[sc])
                    else:
                        nc.vector.tensor_copy(dst, vaccs[sc])
                def vt_group(gq):
                    psv = ps8.tile([128, 512], F16, name="psv", tag="pp")
                    for vt in range(4):
                        skt = gq * 4 + vt
                        nc.tensor.matmul(psv[:, vt * 128:(vt + 1) * 128],
                                         vt_sb[:, skt * 128:(skt + 1) * 128],
                                         iden_sb, is_transpose=True,
                                         start=True, stop=True)
                    nc.vector.tensor_copy(
                        v_sb[:, gq * 4:(gq + 1) * 4, :].rearrange("p a b -> p (a b)"),
                        psv)

                # groups 0,1 (sk chunks 0..7) inline: qc=1 attention needs
                # them immediately; groups 2,3 are deferred filler
                vt_group(0)
                vt_group(1)

            # ------------- phase 2+3: attention with fc FIFO interleaved -------------
            with tc.tile_pool(name="out_sb", bufs=3) as out_sb:
                pending = deque()  # fc / Qh1 / V-transpose filler closures

                def rope_chunk(mh, qc):
                    # in-place rope on qt_sb[:, mh, qc block]; evens in
                    # partitions 0:64, odds in 64:128, via small DVE temps
                    sl = qt_sb[:, mh, qc * 512:(qc + 1) * 512]
                    pe, po = sl[0:64, :], sl[64:128, :]
                    cs, sn = slice(qc * 512, (qc + 1) * 512), slice(qc * 512, (qc + 1) * 512)
                    t1 = es_pool.tile([64, 512], F16, name="t1", tag="rt1", bufs=1)
                    t2 = es_pool.tile([64, 512], F16, name="t2", tag="rt2", bufs=1)
                    nc.vector.tensor_tensor(t1, pe, cos_sb[0:64, cs], op=mybir.AluOpType.mult)
                    nc.vector.tensor_tensor(t2, po, sin_sb[64:128, sn], op=mybir.AluOpType.mult)
                    t3 = es_pool.tile([64, 512], F16, name="t3", tag="rt3", bufs=1)
                    t4 = es_pool.tile([64, 512], F16, name="t4", tag="rt4", bufs=1)
                    nc.vector.tensor_tensor(t3, pe, sin_sb[0:64, sn], op=mybir.AluOpType.mult)
                    nc.vector.tensor_tensor(t4, po, cos_sb[64:128, cs], op=mybir.AluOpType.mult)
                    nc.vector.tensor_tensor(sl[0:64, :], t1, t2, op=mybir.AluOpType.subtract)
                    nc.vector.tensor_tensor(sl[64:128, :], t3, t4, op=mybir.AluOpType.add)

                def qh1_ops(mh, qc):
                    # two filler ops computing qt head mh, block qc: 8+8
                    # accumulation matmuls, then PSUM->SBUF copy + rope
                    psq = []

                    def op_a():
                        psq.append(ps8.tile([128, 512], F32, name="psq", tag="pp"))
                        for k in range(8):
                            nc.tensor.matmul(psq[0], wq_sb[:, k, mh * 128:(mh + 1) * 128],
                                             xt_sb[:, k, qc * 512:(qc + 1) * 512],
                                             start=(k == 0), stop=False)

                    def op_b():
                        for k in range(8, KC):
                            nc.tensor.matmul(psq[0], wq_sb[:, k, mh * 128:(mh + 1) * 128],
                                             xt_sb[:, k, qc * 512:(qc + 1) * 512],
                                             start=False, stop=(k == KC - 1))
                        dst = qt_sb[:, mh, qc * 512:(qc + 1) * 512]
                        if qc % 2 == 0:
                            nc.scalar.copy(dst, psq[0])
                        else:
                            nc.vector.tensor_copy(dst, psq[0])
                        rope_chunk(mh, qc)

                    return [op_a, op_b]

                for mh in (2, 3):
                    for qc in range(SQC):
                        pending.extend(qh1_ops(mh, qc))
                pending.append(lambda: vt_group(2))
                pending.append(lambda: vt_group(3))

                def push_fc(sqt):
                    ob = out_sb.tile([128, D], F16, name="ob", tag="ob")

                    def mk(nf):
                        def op():
                            psf = ps8.tile([128, 512], F32, name="psf", tag="pp")
                            for h2 in range(HG):
                                nc.tensor.matmul(psf,
                                                 ctxT[:, h2, sqt * 128:(sqt + 1) * 128],
                                                 fcw_sb[:, h2, nf * 512:(nf + 1) * 512],
                                                 start=(h2 == 0), stop=(h2 == HG - 1))
                            dst = ob[:, nf * 512:(nf + 1) * 512]
                            if nf % 2 == 0:
                                nc.vector.tensor_copy(dst, psf)
                            else:
                                nc.scalar.copy(dst, psf)
                        return op

                    for nf in range(4):
                        pending.append(mk(nf))
                        if nf % 2 == 1:
                            half = nf // 2
                            pending.append(lambda half=half, ob=ob: nc.sync.dma_start(
                                out=out[sqt * 128:(sqt + 1) * 128,
                                        half * 1024:(half + 1) * 1024],
                                in_=ob[:, half * 1024:(half + 1) * 1024]))

                def scores_op(qc, h, kc):
                    t = kc - 4 * qc
                    pss = ps8.tile([128, 512], F32, name="pss", tag="pp")
                    es = es_pool.tile([128, 512], F16, name="es", tag="es", bufs=8)
                    z = 128 * t if t > 0 else 0  # dead columns on diag tiles
                    qs = qt_sb[:, h, qc * 512:(qc + 1) * 512]
                    nc.tensor.matmul(pss[:, z:512], kt_sb[:, kc * 128:(kc + 1) * 128],
                                     qs[:, z:512], start=True, stop=True)
                    nc.scalar.activation(es[:, z:512], pss[:, z:512],
                                         mybir.ActivationFunctionType.Exp,
                                         scale=SCALE)
                    if t >= 0:
                        # full-width: mask is 0 in the dead columns [0:z], so
                        # this also zeroes the region the exp never wrote
                        # (slot garbage is finite: the ring is primed below)
                        nc.vector.tensor_tensor(es, es, mask_sb[:, t, :],
                                                op=mybir.AluOpType.mult)
                    return es

                # prime the es ring: zero all 6 slots once so the full-width
                # mask multiply never touches uninitialized (possibly NaN)
                # SBUF on the first rotation
                for _ in range(8):
                    es0 = es_pool.tile([128, 512], F16, name="es", tag="es", bufs=8)
                    nc.vector.memset(es0, 0.0)

                QC_ORDER = (1, 2, 3, 0)
                seq = [(qc, h) for qc in QC_ORDER for h in range(HG)]
                prefetched = None

                for i, (qc, h) in enumerate(seq):
                    nkc = 4 * (qc + 1)  # causal: sk chunks 0..nkc-1
                    npairs = nkc // 2
                    ngroups = nkc // 4
                    psc = ps8.tile([128, 512], F32, name="psc", tag="pp")
                    psd = ps8.tile([1, 512], F32, name="psd", tag="pp")
                    es_tiles = [None] * nkc

                    issued = 0
                    if prefetched is not None:
                        for es in prefetched:
                            es_tiles[issued] = es
                            issued += 1
                        prefetched = None

                    def ensure_scores(upto):
                        nonlocal issued
                        while issued < min(upto, nkc):
                            es_tiles[issued] = scores_op(qc, h, issued)
                            issued += 1

                    ensure_scores(4 if i == 0 else 2)

                    # fc-FIFO drain quota: clear the backlog evenly across
                    # the heads of this qc so nothing piles into the tail
                    quota = -(-len(pending) // (HG - h)) if pending else 0
                    drained = 0

                    def dr(n=1):
                        nonlocal drained
                        while drained < quota and n > 0 and pending:
                            pending.popleft()()
                            drained += 1
                            n -= 1

                    def accum_pv(kc):
                        t = kc - 4 * qc
                        z = 128 * t if t > 0 else 0
                        nc.tensor.matmul(psc[:, z:512], v_sb[:, kc, :],
                                         es_tiles[kc][:, z:512],
                                         start=(kc == 0), stop=(kc == nkc - 1))

                    pairs = []
                    groups = []

                    def den_emit(p):
                        esum = es_pool.tile([128, 512], F16, name="esum",
                                            tag="esum", bufs=3)
                        nc.vector.tensor_tensor(esum, es_tiles[2 * p],
                                                es_tiles[2 * p + 1],
                                                op=mybir.AluOpType.add)
                        pairs.append(esum)
                        if len(pairs) == 2:
                            # DVE, not gpsimd: mixing op types on gpsimd
                            # forces ~5us microcode lib swaps per switch
                            e4 = es_pool.tile([128, 512], F16, name="e4",
                                              tag="e4", bufs=2)
                            nc.vector.tensor_tensor(e4, pairs[0], pairs[1],
                                                    op=mybir.AluOpType.add)
                            del pairs[:]
                            groups.append(e4)

                    def den_mm(g):
                        nc.tensor.matmul(psd, ones_sb, groups[g],
                                         start=(g == 0), stop=(g == ngroups - 1))

                    for p in range(npairs):
                        if p + 1 < npairs:
                            ensure_scores(2 * p + 3)
                            dr()
                            ensure_scores(2 * p + 4)
                        accum_pv(2 * p)
                        accum_pv(2 * p + 1)
                        den_emit(p)
                        if p % 2 == 1:
                            g = p // 2
                            if g >= 1:
                                den_mm(g - 1)
                            dr()

                    # finish this head's drain quota BEFORE the prefetch: the
                    # next head's qt block may be produced by a pending
                    # Qh1 filler op, which must be issued first
                    dr(quota - drained)

                    # prefetch the next head's first scores: PE work for
                    # ACT/DVE to chew on while this head's denominator lands
                    if i + 1 < len(seq):
                        nqc, nh = seq[i + 1]
                        depth = 4 if nqc == 0 else 2
                        prefetched = [scores_op(nqc, nh, kc) for kc in range(depth)]

                    den_mm(ngroups - 1)

                    rec = nrm_pool.tile([1, 512], F32, name="rec", tag="rec")
                    nc.vector.reciprocal_approx_fast(rec, psd)
                    rb = nrm_pool.tile([128, 512], F32, name="rb", tag="rb")
                    nc.gpsimd.partition_broadcast(rb, rec)
                    nc.vector.tensor_tensor(ctxT[:, h, qc * 512:(qc + 1) * 512],
                                            psc, rb, op=mybir.AluOpType.mult)

                    if h == HG - 1:
                        for sqt in range(4 * qc, 4 * qc + 4):
                            push_fc(sqt)

                while pending:
                    pending.popleft()()

            nrm_pool.release()
            es_pool.release()
            ps8.release()

    nc.compile()
    return nc


def _get_compiled():
    global _COMPILED
    if _COMPILED is None:
        _COMPILED = _build()
    return _COMPILED


def _prep_inputs(x, w_q, w_kv, fc_w, fc_b, freqs_cos, freqs_sin):
    x = np.asarray(x, dtype=np.float32)
    w_q = np.asarray(w_q, dtype=np.float32)
    w_kv = np.asarray(w_kv, dtype=np.float32)
    fc_w = np.asarray(fc_w, dtype=np.float32)
    freqs_cos = np.asarray(freqs_cos, dtype=np.float32)
    freqs_sin = np.asarray(freqs_sin, dtype=np.float32)

    # rope pair permutation: evens then odds within each head's DK block
    perm = np.concatenate([np.arange(0, DK, 2), np.arange(1, DK, 2)])

    cosT = np.ascontiguousarray(freqs_cos.T).astype(np.float16)  # [64, S]
    sinT = np.ascontiguousarray(freqs_sin.T).astype(np.float16)
    cosT = np.concatenate([cosT, cosT], axis=0)  # duplicate across halves
    sinT = np.concatenate([sinT, sinT], axis=0)

    # masks[i, t, j] = 1 if i <= j - 128*t  (diagonal tiles, t = kc - 4*qc)
    i_idx = np.arange(128)[:, None, None]
    t_idx = np.arange(4)[None, :, None]
    j_idx = np.arange(512)[None, None, :]
    masks = (i_idx <= j_idx - 128 * t_idx).astype(np.float16)
    onesc = np.ones((128, 1), dtype=np.float16)
    iden = np.eye(128, dtype=np.float16)

    in_maps = []
    for c in range(8):
        b, g = divmod(c, 4)
        xT = np.ascontiguousarray(x[b].T).astype(np.float16)
        wq_g = w_q[:, g * HG * DK:(g + 1) * HG * DK].reshape(D, HG, DK)[:, :, perm]
        wq_g = wq_g.reshape(D, HG * DK)
        # pack [2048, 512] -> [128, 4 groups, 4*512]: chunk k rows 128k..128k+127
        # land on partition p = row % 128, 4 chunks concatenated per group row
        wq_g = np.ascontiguousarray(
            wq_g.reshape(4, 4, 128, HG * DK).transpose(2, 0, 1, 3)
                .reshape(128, 4, 4 * HG * DK)).astype(np.float16)
        wk_g = w_kv[:, g * DK:(g + 1) * DK][:, perm]
        wk_g = np.ascontiguousarray(
            wk_g.reshape(KC, 128, DK).transpose(1, 0, 2).reshape(128, KC * DK)
        ).astype(np.float16)
        wv_g = w_kv[:, HKV * DK + g * DK:HKV * DK + (g + 1) * DK]
        wv_g = np.ascontiguousarray(
            wv_g.reshape(KC, 128, DK).transpose(1, 0, 2).reshape(128, KC * DK)
        ).astype(np.float16)
        fcw_g = np.ascontiguousarray(fc_w[g * HG * DK:(g + 1) * HG * DK, :]).astype(np.float16)
        in_maps.append({
            "xT": xT, "wq": wq_g, "wk": wk_g, "wv": wv_g, "fcw": fcw_g,
            "cosT": cosT, "sinT": sinT, "masks": masks, "onesc": onesc,
            "iden": iden,
        })
    return in_maps


_WARMED = False


def kernel_run(trace=False, warmup=True, **inputs):
    global _WARMED
    nc = _get_compiled()
    in_maps = _prep_inputs(**inputs)
    if warmup and not _WARMED:
        # first post-compile execution on a cold device is ~15% slower
        # (table loads / HAM state); do a throwaway run
        run_bass_kernel_spmd(nc, in_maps, core_ids=list(range(8)), trace=False)
        _WARMED = True
    res = run_bass_kernel_spmd(nc, in_maps, core_ids=list(range(8)), trace=trace)
    fc_b = np.asarray(inputs["fc_b"], dtype=np.float32)
    out = np.zeros((B, S, D), dtype=np.float32)
    for c in range(8):
        b = c // 4
        out[b] += res.results[c]["out"].astype(np.float32)
    out += fc_b[None, None, :]
    return out, res


def kernel(**inputs):
    out, _ = kernel_run(trace=False, **inputs)
    return out
